# revision 6
# baseline (speedup 1.0000x reference)
"""MiniBatchDiscrimination Trainium2 kernel (symmetric window).

reference:
    proj = x @ W.T                      # [512, 500] -> [512, 100, 5]
    l1[i,j,o] = sum_k |proj[i,o,k] - proj[j,o,k]|
    mbd[i,o]  = sum_j exp(-l1[i,j,o]) - 1
    out = concat([x, mbd], axis=1)      # [512, 1124]

v3 strategy (8 cores):
  - Symmetry: core r computes its 64 rows against a 320-col cyclic
    window (own block + next 4 blocks; the distance-4 block is computed
    from both sides into own accums only). Reciprocal sums R over
    window cols [64,256) (distance 1-3) accumulate on DVE in fp16; the
    host combines mbd[row] = own_accum + sum_d R_{r-d}[64(d-1)+i] - 1.
  - DVE rows: |d| = 2*relu(d) - d and sum_k d_k telescopes:
    l1 = 2*sum_k relu(d) - Q[o,j] + P_i[o], Q = sum_k projT
    (precomputed). DVE does 4 fused tensor_scalar(subtract, max 0) ops
    per row; PSUM gets 2*sum relu via a 2.0-selector matmul plus one
    identity matmul of -Q; P_i folds into the exp bias.
  - Act rows (balance): ScalarE computes |d| directly via
    Abs(-projT + col) into the quad; 1.0-selector matmul, no Q term,
    zero exp bias.
  - exp + j-reduce: ScalarE Exp(scale=-1, bias) reading PSUM, accum_out
    -> mbdT column, E written fp16 to SBUF for the DVE R-adds.
  - Input DMA in 8 chunks per tensor, overlapped with proj matmuls.
"""

import sys

import numpy as np

sys.path.insert(0, "/opt/trn_rl_repo")

import concourse.bacc as bacc  # noqa: E402
import concourse.mybir as mybir  # noqa: E402
import concourse.tile as tile  # noqa: E402
from concourse.bass_utils import run_bass_kernel_spmd  # noqa: E402

B, IN, O, K = 512, 1024, 100, 5
OK = O * K  # 500
NCORES = 8
BL = B // NCORES  # 64 local rows per core
WIN = 5 * BL  # 320 window cols per core
RLO, RHI = BL, 4 * BL  # reciprocal cols [64, 256)
RW = RHI - RLO  # 192
NT = 4  # proj.T partition tiles
PT = OK // NT  # 125 partitions per tile
NIN = IN // 128  # 8 contraction chunks

F32 = mybir.dt.float32
F16 = mybir.dt.float16
AF = mybir.ActivationFunctionType
ALU = mybir.AluOpType

GSZ = 7  # i-rows per PSUM group (7 banks; 1 bank holds R)
HALF_ROWS = frozenset(i for i in range(BL) if i % 16 == 7)
BS = 32  # sub-block size (16 blocks of 32 over B)
EW = 288  # effective window per row (9 sub-blocks)
RRLO, RRHI = 32, 256  # R region, relative to the row's 288-window
RW2 = RRHI - RRLO  # 224


def build():
    nc = bacc.Bacc("TRN2", target_bir_lowering=False)
    xT_d = nc.dram_tensor("xT", [128, NIN * WIN], F16, kind="ExternalInput")
    wT_d = nc.dram_tensor("wT", [128, NIN * OK], F16, kind="ExternalInput")
    sel2_d = nc.dram_tensor("sel2", [PT, NT, O], F16, kind="ExternalInput")
    sel1_d = nc.dram_tensor("sel1", [PT, NT, O], F16, kind="ExternalInput")
    eye_d = nc.dram_tensor("eye", [O, O], F16, kind="ExternalInput")
    mbdT_d = nc.dram_tensor("mbdT", [O, BL], F32, kind="ExternalOutput")
    r_d = nc.dram_tensor("R", [O, RW], F16, kind="ExternalOutput")

    with tile.TileContext(nc) as tc:
        with (
            tc.tile_pool(name="pers", bufs=1) as pers,
            tc.tile_pool(name="io", bufs=1) as io,
            tc.tile_pool(name="work", bufs=16) as work,
            tc.tile_pool(name="ework", bufs=8) as ework,
            tc.tile_pool(name="ps", bufs=7, space="PSUM") as ps,
            tc.tile_pool(name="rps", bufs=1, space="PSUM") as rps_pool,
        ):
            # ---- input DMA: paired chunks, issue split across sync+scalar
            xcat = io.tile([128, NIN, WIN], F16, name="xcat", tag="xcat")
            wcat = io.tile([128, NIN, OK], F16, name="wcat", tag="wcat")
            for c2 in range(NIN // 2):
                nc.sync.dma_start(
                    out=xcat[:, 2 * c2 : 2 * c2 + 2, :],
                    in_=xT_d[:, 2 * c2 * WIN : (2 * c2 + 2) * WIN],
                )
                nc.sync.dma_start(
                    out=wcat[:, 2 * c2 : 2 * c2 + 2, :],
                    in_=wT_d[:, 2 * c2 * OK : (2 * c2 + 2) * OK],
                )
            s2_all = pers.tile([PT, NT, O], F16, name="s2a", tag="s2a")
            s1_all = pers.tile([PT, NT, O], F16, name="s1a", tag="s1a")
            eye_sb = pers.tile([O, O], F16, name="eye", tag="eye")
            nc.sync.dma_start(out=s2_all[:], in_=sel2_d[:, :, :])
            nc.sync.dma_start(out=s1_all[:], in_=sel1_d[:, :, :])
            nc.sync.dma_start(out=eye_sb[:], in_=eye_d[:, :])
            s2_sb = [s2_all[:, t, :] for t in range(NT)]
            s1_sb = [s1_all[:, t, :] for t in range(NT)]

            projTb = [
                pers.tile([PT, WIN], F16, name=f"projTb{t}", tag=f"projTb{t}")
                for t in range(NT)
            ]
            projL = [
                pers.tile([PT, BL], F32, name=f"projL{t}", tag=f"projL{t}")
                for t in range(NT)
            ]
            qn_sb = pers.tile([O, WIN], F16, name="qn", tag="qn")  # -Q fp16
            pln_sb = pers.tile([O, BL], F32, name="pln", tag="pln")  # -P_i f32
            # subset (tiles 0,1) variants for half-act rows
            qnd_sb = pers.tile([O, WIN], F16, name="qnd", tag="qnd")
            plnd_sb = pers.tile([O, BL], F32, name="plnd", tag="plnd")
            mbdT_sb = pers.tile([O, BL], F32, name="mbdT_sb", tag="mbdT_sb")
            r_sb = pers.tile([O, RW], F16, name="r_sb", tag="r_sb")
            r_ps = rps_pool.tile([O, RW], F32, name="r_ps", tag="r_ps")

            # ---- proj matmuls, chunk-pipelined against the input DMA ----
            pps = [
                ps.tile([PT, WIN], F32, name=f"pps{t}", tag="ps") for t in range(NT)
            ]
            for c in range(NIN):
                for t in range(NT):
                    nc.tensor.matmul(
                        pps[t][:],
                        lhsT=wcat[:, c, PT * t : PT * (t + 1)],
                        rhs=xcat[:, c, :],
                        start=(c == 0),
                        stop=(c == NIN - 1),
                    )
            for t in range(NT):
                nc.vector.tensor_copy(projTb[t][:], pps[t][:])
                nc.scalar.copy(projL[t][:], pps[t][:, :BL])

            # ---- Q = sum_k projT over window; store -Q f16, -P f32 ----
            qps = ps.tile([O, WIN], F32, name="qps", tag="ps")
            for t in range(NT):
                nc.tensor.matmul(
                    qps[:],
                    lhsT=s2_sb[t],
                    rhs=projTb[t][:],
                    start=(t == 0),
                    stop=(t == NT - 1),
                )
            # qps = 2*Q -> qn = -Q (f16), pln = -P (f32, local cols)
            nc.vector.tensor_scalar(qn_sb[:], qps[:], -0.5, None, op0=ALU.mult)
            nc.vector.tensor_scalar(pln_sb[:], qps[:, :BL], -0.5, None, op0=ALU.mult)
            qpsd = ps.tile([O, WIN], F32, name="qpsd", tag="ps")
            for t in range(2):
                nc.tensor.matmul(
                    qpsd[:],
                    lhsT=s2_sb[t],
                    rhs=projTb[t][:],
                    start=(t == 0),
                    stop=(t == 1),
                )
            nc.vector.tensor_scalar(qnd_sb[:], qpsd[:], -0.5, None, op0=ALU.mult)
            nc.vector.tensor_scalar(
                plnd_sb[:], qpsd[:, :BL], -0.5, None, op0=ALU.mult
            )

            # ---- pairwise phase ----
            for g0 in range(0, BL, GSZ):
                gis = list(range(g0, min(g0 + GSZ, BL)))
                half = {i: i in HALF_ROWS for i in gis}
                psums = {
                    i: ps.tile([O, WIN], F32, name=f"ps{i}", tag="ps") for i in gis
                }
                aqs = {}
                for i in gis:
                    aq = work.tile([PT, NT, WIN], F16, name=f"a{i}", tag="A")
                    for t in range(NT):
                        if half[i] and t >= 2:
                            nc.scalar.activation(
                                out=aq[:, t, :],
                                in_=projTb[t][:],
                                func=AF.Abs,
                                bias=projL[t][:, i : i + 1],
                                scale=-1.0,
                            )
                        else:
                            nc.vector.tensor_scalar(
                                aq[:, t, :],
                                projTb[t][:],
                                projL[t][:, i : i + 1],
                                0.0,
                                op0=ALU.subtract,
                                op1=ALU.max,
                            )
                    aqs[i] = aq
                # w-outer matmuls: identical weights back-to-back
                for i in gis:
                    nc.tensor.matmul(
                        psums[i][:],
                        lhsT=eye_sb[:],
                        rhs=(qnd_sb if half[i] else qn_sb)[:],
                        start=True,
                        stop=False,
                    )
                for t in range(NT):
                    for i in gis:
                        sel = s1_sb[t] if (half[i] and t >= 2) else s2_sb[t]
                        nc.tensor.matmul(
                            psums[i][:],
                            lhsT=sel,
                            rhs=aqs[i][:, t, :],
                            start=False,
                            stop=(t == NT - 1),
                        )
                for i in gis:
                    e_i = ework.tile([O, WIN], F16, name=f"e{i}", tag="E")
                    nc.scalar.activation(
                        out=e_i[:],
                        in_=psums[i][:],
                        func=AF.Exp,
                        bias=(plnd_sb if half[i] else pln_sb)[:, i : i + 1],
                        scale=-1.0,
                        accum_out=mbdT_sb[:, i : i + 1],
                    )
                    # R += E[:, RLO:RHI] accumulated on the PE (PSUM bank)
                    nc.tensor.matmul(
                        r_ps[:],
                        lhsT=eye_sb[:],
                        rhs=e_i[:, RLO:RHI],
                        start=(i == 0),
                        stop=(i == BL - 1),
                        skip_group_check=True,
                    )

            nc.vector.tensor_copy(r_sb[:], r_ps[:])
            nc.sync.dma_start(out=mbdT_d[:, :], in_=mbdT_sb[:])
            nc.sync.dma_start(out=r_d[:, :], in_=r_sb[:])
    nc.compile()
    return nc


_CACHE = {}


def _build_cached():
    if "nc" not in _CACHE:
        _CACHE["nc"] = build()
    return _CACHE["nc"]


def _selector(v: float) -> np.ndarray:
    sel = np.zeros((NT, PT, O), np.float32)
    for t in range(NT):
        for p in range(PT):
            sel[t, p, (t * PT + p) % O] = v
    return sel.astype(np.float16)


def make_in_maps(x: np.ndarray, W: np.ndarray):
    xT = np.ascontiguousarray(x.T.astype(np.float16))  # [IN, B]
    # k-major proj.T rows: row p corresponds to (o = p % O, k = p // O)
    perm = np.array([(p % O) * K + p // O for p in range(OK)], np.int64)
    wTk = np.ascontiguousarray(W.T.astype(np.float16)[:, perm])  # [IN, OK]
    sel2 = np.ascontiguousarray(_selector(2.0).transpose(1, 0, 2))
    sel1 = np.ascontiguousarray(_selector(1.0).transpose(1, 0, 2))
    eye = np.eye(O, dtype=np.float16)

    def prep(a, cols):
        return np.ascontiguousarray(
            a.reshape(NIN, 128, cols).transpose(1, 0, 2).reshape(128, NIN * cols)
        )

    wprep = prep(wTk, OK)
    in_maps = []
    for r in range(NCORES):
        cols = (BL * r + np.arange(WIN)) % B
        in_maps.append(
            {
                "xT": prep(np.ascontiguousarray(xT[:, cols]), WIN),
                "wT": wprep,
                "sel2": sel2,
                "sel1": sel1,
                "eye": eye,
            }
        )
    return in_maps


def run(x, W, trace=False, **kw):
    nc = _build_cached()
    in_maps = make_in_maps(x, W)
    return run_bass_kernel_spmd(
        nc, in_maps, core_ids=list(range(NCORES)), trace=trace, **kw
    )


def kernel(x: np.ndarray, W: np.ndarray) -> np.ndarray:
    x = np.asarray(x, np.float32)
    W = np.asarray(W, np.float32)
    res = run(x, W, trace=False)
    mbd = np.empty((B, O), np.float32)
    for r in range(NCORES):
        m = res.results[r]["mbdT"].astype(np.float32).copy()  # [O, BL]
        for d in (1, 2, 3):
            c = (r - d) % NCORES
            m += res.results[c]["R"][:, BL * (d - 1) : BL * d].astype(np.float32)
        mbd[BL * r : BL * (r + 1), :] = m.T
    mbd -= 1.0
    return np.concatenate([x, mbd], axis=1)


# revision 7
# speedup vs baseline: 1.0335x; 1.0335x over previous
"""MiniBatchDiscrimination Trainium2 kernel (symmetric half-window).

reference:
    proj = x @ W.T                      # [512, 500] -> [512, 100, 5]
    l1[i,j,o] = sum_k |proj[i,o,k] - proj[j,o,k]|
    mbd[i,o]  = sum_j exp(-l1[i,j,o]) - 1
    out = concat([x, mbd], axis=1)      # [512, 1124]

Strategy (8 cores, ~80.5us vs 122.8us for the full-j baseline):
  - Symmetry: l1/E are symmetric in (i,j), so core r computes its 64
    i-rows against only a 320-col cyclic j-window (own 64-block + the
    next 4 blocks; the distance-4 block is computed from both sides
    into own accums). Reciprocal sums R over window cols [64,256)
    (distance 1-3) serve the partner blocks; the host combines
    mbd[64r+i] = own_accum + sum_d R_{r-d}[64(d-1)+i] - 1. Work drops
    to 62.5% of the full 512-col pairwise at zero collective cost.
  - Absdiff without a sign-clear pass (walrus rejects fused abs):
    |d| = 2*relu(d) - d, and sum_k d telescopes to P_i[o] - Q[o,j]
    with Q = sum_k projT precomputed. DVE rows do 4 fused
    tensor_scalar(op0=subtract, op1=max 0) ops (2x mode); the PSUM
    gets 2*sum_k relu via a 2.0-selector matmul plus one identity
    matmul of -Q, and P_i folds into the exp bias for free.
  - 8 "half" rows per 64 rebalance DVE->Act: tiles 2,3 of their quad
    run as ScalarE Abs(-projT + col) with a 1.0-selector and a
    tiles-0,1-only Q correction (qnd/plnd).
  - exp + j-reduce: ScalarE Exp(scale=-1, bias=-P_i) reading PSUM,
    accum_out -> mbdT column; E written fp16 to SBUF.
  - R accumulation on the PE: identity-matmul of E[:, 64:256] into a
    dedicated PSUM bank (start at i=0, stop at i=63), freeing DVE.
  - Input DMA as 2-chunk pairs issued early from the Sync queue and
    overlapped with the projection matmuls; selectors sent host-side
    transposed ([PT, NT, O]) so each loads in one descriptor sweep.
  - GpSimd is left idle on purpose: its Q7 tensor ops are ~10x slower
    than the cost model suggests and their SBUF traffic slows every
    other engine (measured 222us with subs offloaded there).
"""

import sys

import numpy as np

sys.path.insert(0, "/opt/trn_rl_repo")

import concourse.bacc as bacc  # noqa: E402
import concourse.mybir as mybir  # noqa: E402
import concourse.tile as tile  # noqa: E402
from concourse.bass_utils import run_bass_kernel_spmd  # noqa: E402

B, IN, O, K = 512, 1024, 100, 5
OK = O * K  # 500
NCORES = 8
BL = B // NCORES  # 64 local rows per core
WIN = 5 * BL  # 320 window cols per core
RLO, RHI = BL, 4 * BL  # reciprocal cols [64, 256)
RW = RHI - RLO  # 192
NT = 4  # proj.T partition tiles
PT = OK // NT  # 125 partitions per tile
NIN = IN // 128  # 8 contraction chunks

F32 = mybir.dt.float32
F16 = mybir.dt.float16
AF = mybir.ActivationFunctionType
ALU = mybir.AluOpType

GSZ = 7  # i-rows per PSUM group (7 banks; 1 bank holds R)
HALF_ROWS = frozenset(i for i in range(BL) if i % 8 == 3)
BS = 32  # sub-block size (16 blocks of 32 over B)
EW = 288  # effective window per row (9 sub-blocks)
RRLO, RRHI = 32, 256  # R region, relative to the row's 288-window
RW2 = RRHI - RRLO  # 224


def build():
    nc = bacc.Bacc("TRN2", target_bir_lowering=False)
    xT_d = nc.dram_tensor("xT", [128, NIN * WIN], F16, kind="ExternalInput")
    wT_d = nc.dram_tensor("wT", [128, NIN * OK], F16, kind="ExternalInput")
    sel2_d = nc.dram_tensor("sel2", [PT, NT, O], F16, kind="ExternalInput")
    sel1_d = nc.dram_tensor("sel1", [PT, NT, O], F16, kind="ExternalInput")
    eye_d = nc.dram_tensor("eye", [O, O], F16, kind="ExternalInput")
    mbdT_d = nc.dram_tensor("mbdT", [O, BL], F32, kind="ExternalOutput")
    r_d = nc.dram_tensor("R", [O, RW], F16, kind="ExternalOutput")

    with tile.TileContext(nc) as tc:
        with (
            tc.tile_pool(name="pers", bufs=1) as pers,
            tc.tile_pool(name="io", bufs=1) as io,
            tc.tile_pool(name="work", bufs=16) as work,
            tc.tile_pool(name="ework", bufs=8) as ework,
            tc.tile_pool(name="ps", bufs=7, space="PSUM") as ps,
            tc.tile_pool(name="rps", bufs=1, space="PSUM") as rps_pool,
        ):
            # ---- input DMA: paired chunks, issue split across sync+scalar
            xcat = io.tile([128, NIN, WIN], F16, name="xcat", tag="xcat")
            wcat = io.tile([128, NIN, OK], F16, name="wcat", tag="wcat")
            for c2 in range(NIN // 2):
                nc.sync.dma_start(
                    out=xcat[:, 2 * c2 : 2 * c2 + 2, :],
                    in_=xT_d[:, 2 * c2 * WIN : (2 * c2 + 2) * WIN],
                )
                nc.sync.dma_start(
                    out=wcat[:, 2 * c2 : 2 * c2 + 2, :],
                    in_=wT_d[:, 2 * c2 * OK : (2 * c2 + 2) * OK],
                )
            s2_all = pers.tile([PT, NT, O], F16, name="s2a", tag="s2a")
            s1_all = pers.tile([PT, NT, O], F16, name="s1a", tag="s1a")
            eye_sb = pers.tile([O, O], F16, name="eye", tag="eye")
            nc.sync.dma_start(out=s2_all[:], in_=sel2_d[:, :, :])
            nc.sync.dma_start(out=s1_all[:], in_=sel1_d[:, :, :])
            nc.sync.dma_start(out=eye_sb[:], in_=eye_d[:, :])
            s2_sb = [s2_all[:, t, :] for t in range(NT)]
            s1_sb = [s1_all[:, t, :] for t in range(NT)]

            projTb = [
                pers.tile([PT, WIN], F16, name=f"projTb{t}", tag=f"projTb{t}")
                for t in range(NT)
            ]
            projL = [
                pers.tile([PT, BL], F32, name=f"projL{t}", tag=f"projL{t}")
                for t in range(NT)
            ]
            qn_sb = pers.tile([O, WIN], F16, name="qn", tag="qn")  # -Q fp16
            pln_sb = pers.tile([O, BL], F32, name="pln", tag="pln")  # -P_i f32
            # subset (tiles 0,1) variants for half-act rows
            qnd_sb = pers.tile([O, WIN], F16, name="qnd", tag="qnd")
            plnd_sb = pers.tile([O, BL], F32, name="plnd", tag="plnd")
            mbdT_sb = pers.tile([O, BL], F32, name="mbdT_sb", tag="mbdT_sb")
            r_sb = pers.tile([O, RW], F16, name="r_sb", tag="r_sb")
            r_ps = rps_pool.tile([O, RW], F32, name="r_ps", tag="r_ps")

            # ---- proj matmuls, chunk-pipelined against the input DMA ----
            pps = [
                ps.tile([PT, WIN], F32, name=f"pps{t}", tag="ps") for t in range(NT)
            ]
            for c in range(NIN):
                for t in range(NT):
                    nc.tensor.matmul(
                        pps[t][:],
                        lhsT=wcat[:, c, PT * t : PT * (t + 1)],
                        rhs=xcat[:, c, :],
                        start=(c == 0),
                        stop=(c == NIN - 1),
                    )
            for t in range(NT):
                nc.vector.tensor_copy(projTb[t][:], pps[t][:])
                nc.scalar.copy(projL[t][:], pps[t][:, :BL])

            # ---- Q = sum_k projT over window; store -Q f16, -P f32 ----
            qps = ps.tile([O, WIN], F32, name="qps", tag="ps")
            for t in range(NT):
                nc.tensor.matmul(
                    qps[:],
                    lhsT=s2_sb[t],
                    rhs=projTb[t][:],
                    start=(t == 0),
                    stop=(t == NT - 1),
                )
            # qps = 2*Q -> qn = -Q (f16), pln = -P (f32, local cols)
            nc.vector.tensor_scalar(qn_sb[:], qps[:], -0.5, None, op0=ALU.mult)
            nc.vector.tensor_scalar(pln_sb[:], qps[:, :BL], -0.5, None, op0=ALU.mult)
            qpsd = ps.tile([O, WIN], F32, name="qpsd", tag="ps")
            for t in range(2):
                nc.tensor.matmul(
                    qpsd[:],
                    lhsT=s2_sb[t],
                    rhs=projTb[t][:],
                    start=(t == 0),
                    stop=(t == 1),
                )
            nc.vector.tensor_scalar(qnd_sb[:], qpsd[:], -0.5, None, op0=ALU.mult)
            nc.vector.tensor_scalar(
                plnd_sb[:], qpsd[:, :BL], -0.5, None, op0=ALU.mult
            )

            # ---- pairwise phase ----
            for g0 in range(0, BL, GSZ):
                gis = list(range(g0, min(g0 + GSZ, BL)))
                half = {i: i in HALF_ROWS for i in gis}
                psums = {
                    i: ps.tile([O, WIN], F32, name=f"ps{i}", tag="ps") for i in gis
                }
                aqs = {}
                for i in gis:
                    aq = work.tile([PT, NT, WIN], F16, name=f"a{i}", tag="A")
                    for t in range(NT):
                        if half[i] and t >= 2:
                            nc.scalar.activation(
                                out=aq[:, t, :],
                                in_=projTb[t][:],
                                func=AF.Abs,
                                bias=projL[t][:, i : i + 1],
                                scale=-1.0,
                            )
                        else:
                            nc.vector.tensor_scalar(
                                aq[:, t, :],
                                projTb[t][:],
                                projL[t][:, i : i + 1],
                                0.0,
                                op0=ALU.subtract,
                                op1=ALU.max,
                            )
                    aqs[i] = aq
                # w-outer matmuls: identical weights back-to-back
                for i in gis:
                    nc.tensor.matmul(
                        psums[i][:],
                        lhsT=eye_sb[:],
                        rhs=(qnd_sb if half[i] else qn_sb)[:],
                        start=True,
                        stop=False,
                    )
                for t in range(NT):
                    for i in gis:
                        sel = s1_sb[t] if (half[i] and t >= 2) else s2_sb[t]
                        nc.tensor.matmul(
                            psums[i][:],
                            lhsT=sel,
                            rhs=aqs[i][:, t, :],
                            start=False,
                            stop=(t == NT - 1),
                        )
                for i in gis:
                    e_i = ework.tile([O, WIN], F16, name=f"e{i}", tag="E")
                    nc.scalar.activation(
                        out=e_i[:],
                        in_=psums[i][:],
                        func=AF.Exp,
                        bias=(plnd_sb if half[i] else pln_sb)[:, i : i + 1],
                        scale=-1.0,
                        accum_out=mbdT_sb[:, i : i + 1],
                    )
                    # R += E[:, RLO:RHI] accumulated on the PE (PSUM bank)
                    nc.tensor.matmul(
                        r_ps[:],
                        lhsT=eye_sb[:],
                        rhs=e_i[:, RLO:RHI],
                        start=(i == 0),
                        stop=(i == BL - 1),
                        skip_group_check=True,
                    )

            nc.vector.tensor_copy(r_sb[:], r_ps[:])
            nc.sync.dma_start(out=mbdT_d[:, :], in_=mbdT_sb[:])
            nc.sync.dma_start(out=r_d[:, :], in_=r_sb[:])
    nc.compile()
    return nc


_CACHE = {}


def _build_cached():
    if "nc" not in _CACHE:
        _CACHE["nc"] = build()
    return _CACHE["nc"]


def _selector(v: float) -> np.ndarray:
    sel = np.zeros((NT, PT, O), np.float32)
    for t in range(NT):
        for p in range(PT):
            sel[t, p, (t * PT + p) % O] = v
    return sel.astype(np.float16)


def make_in_maps(x: np.ndarray, W: np.ndarray):
    xT = np.ascontiguousarray(x.T.astype(np.float16))  # [IN, B]
    # k-major proj.T rows: row p corresponds to (o = p % O, k = p // O)
    perm = np.array([(p % O) * K + p // O for p in range(OK)], np.int64)
    wTk = np.ascontiguousarray(W.T.astype(np.float16)[:, perm])  # [IN, OK]
    sel2 = np.ascontiguousarray(_selector(2.0).transpose(1, 0, 2))
    sel1 = np.ascontiguousarray(_selector(1.0).transpose(1, 0, 2))
    eye = np.eye(O, dtype=np.float16)

    def prep(a, cols):
        return np.ascontiguousarray(
            a.reshape(NIN, 128, cols).transpose(1, 0, 2).reshape(128, NIN * cols)
        )

    wprep = prep(wTk, OK)
    in_maps = []
    for r in range(NCORES):
        cols = (BL * r + np.arange(WIN)) % B
        in_maps.append(
            {
                "xT": prep(np.ascontiguousarray(xT[:, cols]), WIN),
                "wT": wprep,
                "sel2": sel2,
                "sel1": sel1,
                "eye": eye,
            }
        )
    return in_maps


def run(x, W, trace=False, **kw):
    nc = _build_cached()
    in_maps = make_in_maps(x, W)
    return run_bass_kernel_spmd(
        nc, in_maps, core_ids=list(range(NCORES)), trace=trace, **kw
    )


def kernel(x: np.ndarray, W: np.ndarray) -> np.ndarray:
    x = np.asarray(x, np.float32)
    W = np.asarray(W, np.float32)
    res = run(x, W, trace=False)
    mbd = np.empty((B, O), np.float32)
    for r in range(NCORES):
        m = res.results[r]["mbdT"].astype(np.float32).copy()  # [O, BL]
        for d in (1, 2, 3):
            c = (r - d) % NCORES
            m += res.results[c]["R"][:, BL * (d - 1) : BL * d].astype(np.float32)
        mbd[BL * r : BL * (r + 1), :] = m.T
    mbd -= 1.0
    return np.concatenate([x, mbd], axis=1)


# revision 8
# speedup vs baseline: 1.0619x; 1.0275x over previous
"""MiniBatchDiscrimination Trainium2 kernel (symmetric half-window).

reference:
    proj = x @ W.T                      # [512, 500] -> [512, 100, 5]
    l1[i,j,o] = sum_k |proj[i,o,k] - proj[j,o,k]|
    mbd[i,o]  = sum_j exp(-l1[i,j,o]) - 1
    out = concat([x, mbd], axis=1)      # [512, 1124]

Strategy (8 cores, ~80.5us vs 122.8us for the full-j baseline):
  - Symmetry: l1/E are symmetric in (i,j), so core r computes its 64
    i-rows against only a 320-col cyclic j-window (own 64-block + the
    next 4 blocks; the distance-4 block is computed from both sides
    into own accums). Reciprocal sums R over window cols [64,256)
    (distance 1-3) serve the partner blocks; the host combines
    mbd[64r+i] = own_accum + sum_d R_{r-d}[64(d-1)+i] - 1. Work drops
    to 62.5% of the full 512-col pairwise at zero collective cost.
  - Absdiff without a sign-clear pass (walrus rejects fused abs):
    |d| = 2*relu(d) - d, and sum_k d telescopes to P_i[o] - Q[o,j]
    with Q = sum_k projT precomputed. DVE rows do 4 fused
    tensor_scalar(op0=subtract, op1=max 0) ops (2x mode); the PSUM
    gets 2*sum_k relu via a 2.0-selector matmul plus one identity
    matmul of -Q, and P_i folds into the exp bias for free.
  - 8 "half" rows per 64 rebalance DVE->Act: tiles 2,3 of their quad
    run as ScalarE Abs(-projT + col) with a 1.0-selector and a
    tiles-0,1-only Q correction (qnd/plnd).
  - exp + j-reduce: ScalarE Exp(scale=-1, bias=-P_i) reading PSUM,
    accum_out -> mbdT column; E written fp16 to SBUF.
  - R accumulation on the PE: identity-matmul of E[:, 64:256] into a
    dedicated PSUM bank (start at i=0, stop at i=63), freeing DVE.
  - Input DMA as 2-chunk pairs issued early from the Sync queue and
    overlapped with the projection matmuls; selectors sent host-side
    transposed ([PT, NT, O]) so each loads in one descriptor sweep.
  - GpSimd is left idle on purpose: its Q7 tensor ops are ~10x slower
    than the cost model suggests and their SBUF traffic slows every
    other engine (measured 222us with subs offloaded there).
"""

import sys

import numpy as np

sys.path.insert(0, "/opt/trn_rl_repo")

import concourse.bacc as bacc  # noqa: E402
import concourse.mybir as mybir  # noqa: E402
import concourse.tile as tile  # noqa: E402
from concourse.bass_utils import run_bass_kernel_spmd  # noqa: E402

B, IN, O, K = 512, 1024, 100, 5
OK = O * K  # 500
NCORES = 8
BL = B // NCORES  # 64 local rows per core
WIN = 5 * BL  # 320 window cols per core
RLO, RHI = BL, 4 * BL  # reciprocal cols [64, 256)
RW = RHI - RLO  # 192
NT = 4  # proj.T partition tiles
PT = OK // NT  # 125 partitions per tile
NIN = IN // 128  # 8 contraction chunks

F32 = mybir.dt.float32
F16 = mybir.dt.float16
AF = mybir.ActivationFunctionType
ALU = mybir.AluOpType

GSZ = 7  # i-rows per PSUM group (7 banks; 1 bank holds R)
HALF_ROWS = frozenset(i for i in range(BL) if i % 8 == 3)
BS = 32  # sub-block size (16 blocks of 32 over B)
EW = 288  # effective window per row (9 sub-blocks)
RRLO, RRHI = 32, 256  # R region, relative to the row's 288-window
RW2 = RRHI - RRLO  # 224


def build():
    nc = bacc.Bacc("TRN2", target_bir_lowering=False)
    xT_d = nc.dram_tensor("xT", [128, NIN * WIN], F16, kind="ExternalInput")
    wT_d = nc.dram_tensor("wT", [128, NIN * OK], F16, kind="ExternalInput")
    sel2_d = nc.dram_tensor("sel2", [PT, NT, O], F16, kind="ExternalInput")
    sel1_d = nc.dram_tensor("sel1", [PT, NT, O], F16, kind="ExternalInput")
    eye_d = nc.dram_tensor("eye", [O, O], F16, kind="ExternalInput")
    mbdT_d = nc.dram_tensor("mbdT", [O, BL], F32, kind="ExternalOutput")
    r_d = nc.dram_tensor("R", [O, 2 * RW2], F16, kind="ExternalOutput")

    with tile.TileContext(nc) as tc:
        with (
            tc.tile_pool(name="pers", bufs=1) as pers,
            tc.tile_pool(name="io", bufs=1) as io,
            tc.tile_pool(name="work", bufs=16) as work,
            tc.tile_pool(name="ework", bufs=8) as ework,
            tc.tile_pool(name="ps", bufs=7, space="PSUM") as ps,
            tc.tile_pool(name="rps", bufs=1, space="PSUM") as rps_pool,
        ):
            # ---- input DMA: paired chunks, issue split across sync+scalar
            xcat = io.tile([128, NIN, WIN], F16, name="xcat", tag="xcat")
            wcat = io.tile([128, NIN, OK], F16, name="wcat", tag="wcat")
            for c2 in range(NIN // 2):
                nc.sync.dma_start(
                    out=xcat[:, 2 * c2 : 2 * c2 + 2, :],
                    in_=xT_d[:, 2 * c2 * WIN : (2 * c2 + 2) * WIN],
                )
                nc.sync.dma_start(
                    out=wcat[:, 2 * c2 : 2 * c2 + 2, :],
                    in_=wT_d[:, 2 * c2 * OK : (2 * c2 + 2) * OK],
                )
            s2_all = pers.tile([PT, NT, O], F16, name="s2a", tag="s2a")
            s1_all = pers.tile([PT, NT, O], F16, name="s1a", tag="s1a")
            eye_sb = pers.tile([O, O], F16, name="eye", tag="eye")
            nc.sync.dma_start(out=s2_all[:], in_=sel2_d[:, :, :])
            nc.sync.dma_start(out=s1_all[:], in_=sel1_d[:, :, :])
            nc.sync.dma_start(out=eye_sb[:], in_=eye_d[:, :])
            s2_sb = [s2_all[:, t, :] for t in range(NT)]
            s1_sb = [s1_all[:, t, :] for t in range(NT)]

            projTb = [
                pers.tile([PT, WIN], F16, name=f"projTb{t}", tag=f"projTb{t}")
                for t in range(NT)
            ]
            projL = [
                pers.tile([PT, BL], F32, name=f"projL{t}", tag=f"projL{t}")
                for t in range(NT)
            ]
            qn_sb = pers.tile([O, WIN], F16, name="qn", tag="qn")  # -Q fp16
            pln_sb = pers.tile([O, BL], F32, name="pln", tag="pln")  # -P_i f32
            # subset (tiles 0,1) variants for half-act rows
            qnd_sb = pers.tile([O, WIN], F16, name="qnd", tag="qnd")
            plnd_sb = pers.tile([O, BL], F32, name="plnd", tag="plnd")
            mbdT_sb = pers.tile([O, BL], F32, name="mbdT_sb", tag="mbdT_sb")
            r_sb = pers.tile([O, 2 * RW2], F16, name="r_sb", tag="r_sb")
            r_ps = rps_pool.tile([O, 2, RW2], F32, name="r_ps", tag="r_ps")

            # ---- proj matmuls, chunk-pipelined against the input DMA ----
            pps = [
                ps.tile([PT, WIN], F32, name=f"pps{t}", tag="ps") for t in range(NT)
            ]
            for c in range(NIN):
                for t in range(NT):
                    nc.tensor.matmul(
                        pps[t][:],
                        lhsT=wcat[:, c, PT * t : PT * (t + 1)],
                        rhs=xcat[:, c, :],
                        start=(c == 0),
                        stop=(c == NIN - 1),
                    )
            for t in range(NT):
                nc.vector.tensor_copy(projTb[t][:], pps[t][:])
                nc.vector.tensor_copy(projL[t][:], pps[t][:, :BL])

            # ---- Q = sum_k projT over window; store -Q f16, -P f32 ----
            qps = ps.tile([O, WIN], F32, name="qps", tag="ps")
            for t in range(NT):
                nc.tensor.matmul(
                    qps[:],
                    lhsT=s2_sb[t],
                    rhs=projTb[t][:],
                    start=(t == 0),
                    stop=(t == NT - 1),
                )
            # qps = 2*Q -> qn = -Q (f16), pln = -P (f32, local cols)
            nc.vector.tensor_scalar(qn_sb[:], qps[:], -0.5, None, op0=ALU.mult)
            nc.vector.tensor_scalar(pln_sb[:], qps[:, :BL], -0.5, None, op0=ALU.mult)
            qpsd = ps.tile([O, WIN], F32, name="qpsd", tag="ps")
            for t in range(2):
                nc.tensor.matmul(
                    qpsd[:],
                    lhsT=s2_sb[t],
                    rhs=projTb[t][:],
                    start=(t == 0),
                    stop=(t == 1),
                )
            nc.vector.tensor_scalar(qnd_sb[:], qpsd[:], -0.5, None, op0=ALU.mult)
            nc.vector.tensor_scalar(
                plnd_sb[:], qpsd[:, :BL], -0.5, None, op0=ALU.mult
            )

            # ---- pairwise phase ----
            for g0 in range(0, BL, GSZ):
                gis = list(range(g0, min(g0 + GSZ, BL)))
                half = {i: i in HALF_ROWS for i in gis}
                offs = {i: BS * (i // BS) for i in gis}
                psums = {
                    i: ps.tile([O, EW], F32, name=f"ps{i}", tag="ps") for i in gis
                }
                aqs = {}
                for i in gis:
                    aq = work.tile([PT, NT, EW], F16, name=f"a{i}", tag="A")
                    off = offs[i]
                    for t in range(NT):
                        if half[i] and t >= 2:
                            nc.scalar.activation(
                                out=aq[:, t, :],
                                in_=projTb[t][:, off : off + EW],
                                func=AF.Abs,
                                bias=projL[t][:, i : i + 1],
                                scale=-1.0,
                            )
                        else:
                            nc.vector.tensor_scalar(
                                aq[:, t, :],
                                projTb[t][:, off : off + EW],
                                projL[t][:, i : i + 1],
                                0.0,
                                op0=ALU.subtract,
                                op1=ALU.max,
                            )
                    aqs[i] = aq
                # w-outer matmuls: identical weights back-to-back
                for i in gis:
                    nc.tensor.matmul(
                        psums[i][:],
                        lhsT=eye_sb[:],
                        rhs=(qnd_sb if half[i] else qn_sb)[
                            :, offs[i] : offs[i] + EW
                        ],
                        start=True,
                        stop=False,
                    )
                for t in range(NT):
                    for i in gis:
                        sel = s1_sb[t] if (half[i] and t >= 2) else s2_sb[t]
                        nc.tensor.matmul(
                            psums[i][:],
                            lhsT=sel,
                            rhs=aqs[i][:, t, :],
                            start=False,
                            stop=(t == NT - 1),
                        )
                for i in gis:
                    e_i = ework.tile([O, EW], F16, name=f"e{i}", tag="E")
                    nc.scalar.activation(
                        out=e_i[:],
                        in_=psums[i][:],
                        func=AF.Exp,
                        bias=(plnd_sb if half[i] else pln_sb)[:, i : i + 1],
                        scale=-1.0,
                        accum_out=mbdT_sb[:, i : i + 1],
                    )
                    # R[cls] += E[:, RRLO:RRHI] accumulated on the PE
                    cls = i // BS
                    nc.tensor.matmul(
                        r_ps[:, cls, :],
                        lhsT=eye_sb[:],
                        rhs=e_i[:, RRLO:RRHI],
                        start=(i % BS == 0),
                        stop=(i % BS == BS - 1),
                        skip_group_check=True,
                    )

            nc.vector.tensor_copy(r_sb[:], r_ps[:])
            nc.sync.dma_start(out=mbdT_d[:, :], in_=mbdT_sb[:])
            nc.sync.dma_start(out=r_d[:, :], in_=r_sb[:])
    nc.compile()
    return nc


_CACHE = {}


def _build_cached():
    if "nc" not in _CACHE:
        _CACHE["nc"] = build()
    return _CACHE["nc"]


def _selector(v: float) -> np.ndarray:
    sel = np.zeros((NT, PT, O), np.float32)
    for t in range(NT):
        for p in range(PT):
            sel[t, p, (t * PT + p) % O] = v
    return sel.astype(np.float16)


def make_in_maps(x: np.ndarray, W: np.ndarray):
    xT = np.ascontiguousarray(x.T.astype(np.float16))  # [IN, B]
    # k-major proj.T rows: row p corresponds to (o = p % O, k = p // O)
    perm = np.array([(p % O) * K + p // O for p in range(OK)], np.int64)
    wTk = np.ascontiguousarray(W.T.astype(np.float16)[:, perm])  # [IN, OK]
    sel2 = np.ascontiguousarray(_selector(2.0).transpose(1, 0, 2))
    sel1 = np.ascontiguousarray(_selector(1.0).transpose(1, 0, 2))
    eye = np.eye(O, dtype=np.float16)

    def prep(a, cols):
        return np.ascontiguousarray(
            a.reshape(NIN, 128, cols).transpose(1, 0, 2).reshape(128, NIN * cols)
        )

    wprep = prep(wTk, OK)
    in_maps = []
    for r in range(NCORES):
        cols = (BL * r + np.arange(WIN)) % B
        in_maps.append(
            {
                "xT": prep(np.ascontiguousarray(xT[:, cols]), WIN),
                "wT": wprep,
                "sel2": sel2,
                "sel1": sel1,
                "eye": eye,
            }
        )
    return in_maps


def run(x, W, trace=False, **kw):
    nc = _build_cached()
    in_maps = make_in_maps(x, W)
    return run_bass_kernel_spmd(
        nc, in_maps, core_ids=list(range(NCORES)), trace=trace, **kw
    )


def kernel(x: np.ndarray, W: np.ndarray) -> np.ndarray:
    x = np.asarray(x, np.float32)
    W = np.asarray(W, np.float32)
    res = run(x, W, trace=False)
    accums = [res.results[r]["mbdT"].astype(np.float32) for r in range(NCORES)]
    Rs = [
        res.results[r]["R"].astype(np.float32).reshape(O, 2, RW2)
        for r in range(NCORES)
    ]
    mbd = np.empty((B, O), np.float32)
    NB = B // BS  # 16 sub-blocks
    for m in range(NB):
        r, cls = m // 2, m % 2
        s = accums[r][:, BS * cls : BS * (cls + 1)].copy()  # [O, BS]
        for d in range(1, 8):
            c = (m - d) % NB
            rc, cc = c // 2, c % 2
            s += Rs[rc][:, cc, BS * (d - 1) : BS * d]
        mbd[BS * m : BS * (m + 1), :] = s.T
    mbd -= 1.0
    return np.concatenate([x, mbd], axis=1)


# revision 9
# speedup vs baseline: 1.0715x; 1.0091x over previous
"""MiniBatchDiscrimination Trainium2 kernel (symmetric half-window).

reference:
    proj = x @ W.T                      # [512, 500] -> [512, 100, 5]
    l1[i,j,o] = sum_k |proj[i,o,k] - proj[j,o,k]|
    mbd[i,o]  = sum_j exp(-l1[i,j,o]) - 1
    out = concat([x, mbd], axis=1)      # [512, 1124]

Strategy (8 cores, ~80.5us vs 122.8us for the full-j baseline):
  - Symmetry: l1/E are symmetric in (i,j), so core r computes its 64
    i-rows against only a 320-col cyclic j-window (own 64-block + the
    next 4 blocks; the distance-4 block is computed from both sides
    into own accums). Reciprocal sums R over window cols [64,256)
    (distance 1-3) serve the partner blocks; the host combines
    mbd[64r+i] = own_accum + sum_d R_{r-d}[64(d-1)+i] - 1. Work drops
    to 62.5% of the full 512-col pairwise at zero collective cost.
  - Absdiff without a sign-clear pass (walrus rejects fused abs):
    |d| = 2*relu(d) - d, and sum_k d telescopes to P_i[o] - Q[o,j]
    with Q = sum_k projT precomputed. DVE rows do 4 fused
    tensor_scalar(op0=subtract, op1=max 0) ops (2x mode); the PSUM
    gets 2*sum_k relu via a 2.0-selector matmul plus one identity
    matmul of -Q, and P_i folds into the exp bias for free.
  - 8 "half" rows per 64 rebalance DVE->Act: tiles 2,3 of their quad
    run as ScalarE Abs(-projT + col) with a 1.0-selector and a
    tiles-0,1-only Q correction (qnd/plnd).
  - exp + j-reduce: ScalarE Exp(scale=-1, bias=-P_i) reading PSUM,
    accum_out -> mbdT column; E written fp16 to SBUF.
  - R accumulation on the PE: identity-matmul of E[:, 64:256] into a
    dedicated PSUM bank (start at i=0, stop at i=63), freeing DVE.
  - Input DMA as 2-chunk pairs issued early from the Sync queue and
    overlapped with the projection matmuls; selectors sent host-side
    transposed ([PT, NT, O]) so each loads in one descriptor sweep.
  - GpSimd is left idle on purpose: its Q7 tensor ops are ~10x slower
    than the cost model suggests and their SBUF traffic slows every
    other engine (measured 222us with subs offloaded there).
"""

import sys

import numpy as np

sys.path.insert(0, "/opt/trn_rl_repo")

import concourse.bacc as bacc  # noqa: E402
import concourse.mybir as mybir  # noqa: E402
import concourse.tile as tile  # noqa: E402
from concourse.bass_utils import run_bass_kernel_spmd  # noqa: E402

B, IN, O, K = 512, 1024, 100, 5
OK = O * K  # 500
NCORES = 8
BL = B // NCORES  # 64 local rows per core
WIN = 5 * BL  # 320 window cols per core
RLO, RHI = BL, 4 * BL  # reciprocal cols [64, 256)
RW = RHI - RLO  # 192
NT = 4  # proj.T partition tiles
PT = OK // NT  # 125 partitions per tile
NIN = IN // 128  # 8 contraction chunks

F32 = mybir.dt.float32
F16 = mybir.dt.float16
AF = mybir.ActivationFunctionType
ALU = mybir.AluOpType

GSZ = 7  # i-rows per PSUM group (7 banks; 1 bank holds R)
HALF_ROWS = frozenset(i for i in range(BL) if i % 8 == 3)
BS = 32  # sub-block size (16 blocks of 32 over B)
EW = 288  # effective window per row (9 sub-blocks)
RRLO, RRHI = 32, 256  # R region, relative to the row's 288-window
RW2 = RRHI - RRLO  # 224


def build():
    nc = bacc.Bacc("TRN2", target_bir_lowering=False)
    xT_d = nc.dram_tensor("xT", [128, NIN * WIN], F16, kind="ExternalInput")
    wT_d = nc.dram_tensor("wT", [128, NIN * OK], F16, kind="ExternalInput")
    sel2_d = nc.dram_tensor("sel2", [PT, NT, O], F16, kind="ExternalInput")
    sel1_d = nc.dram_tensor("sel1", [PT, NT, O], F16, kind="ExternalInput")
    eye_d = nc.dram_tensor("eye", [O, O], F16, kind="ExternalInput")
    mbdT_d = nc.dram_tensor("mbdT", [O, BL], F32, kind="ExternalOutput")
    r_d = nc.dram_tensor("R", [O, 2 * RW2], F16, kind="ExternalOutput")

    with tile.TileContext(nc) as tc:
        with (
            tc.tile_pool(name="pers", bufs=1) as pers,
            tc.tile_pool(name="io", bufs=1) as io,
            tc.tile_pool(name="work", bufs=16) as work,
            tc.tile_pool(name="ework", bufs=8) as ework,
            tc.tile_pool(name="ps", bufs=7, space="PSUM") as ps,
            tc.tile_pool(name="rps", bufs=1, space="PSUM") as rps_pool,
        ):
            # ---- input DMA: paired chunks, issue split across sync+scalar
            xcat = io.tile([128, NIN, WIN], F16, name="xcat", tag="xcat")
            wcat = io.tile([128, NIN, OK], F16, name="wcat", tag="wcat")
            for c2 in range(NIN // 2):
                nc.sync.dma_start(
                    out=xcat[:, 2 * c2 : 2 * c2 + 2, :],
                    in_=xT_d[:, 2 * c2 * WIN : (2 * c2 + 2) * WIN],
                )
                nc.sync.dma_start(
                    out=wcat[:, 2 * c2 : 2 * c2 + 2, :],
                    in_=wT_d[:, 2 * c2 * OK : (2 * c2 + 2) * OK],
                )
            s2_all = pers.tile([PT, NT, O], F16, name="s2a", tag="s2a")
            s1_all = pers.tile([PT, NT, O], F16, name="s1a", tag="s1a")
            eye_sb = pers.tile([O, O], F16, name="eye", tag="eye")
            nc.sync.dma_start(out=s2_all[:], in_=sel2_d[:, :, :])
            nc.sync.dma_start(out=s1_all[:], in_=sel1_d[:, :, :])
            nc.sync.dma_start(out=eye_sb[:], in_=eye_d[:, :])
            s2_sb = [s2_all[:, t, :] for t in range(NT)]
            s1_sb = [s1_all[:, t, :] for t in range(NT)]

            projTb = [
                pers.tile([PT, WIN], F16, name=f"projTb{t}", tag=f"projTb{t}")
                for t in range(NT)
            ]
            projL = [
                pers.tile([PT, BL], F32, name=f"projL{t}", tag=f"projL{t}")
                for t in range(NT)
            ]
            qn_sb = pers.tile([O, WIN], F16, name="qn", tag="qn")  # -Q fp16
            pln_sb = pers.tile([O, BL], F32, name="pln", tag="pln")  # -P_i f32
            # subset (tiles 0,1) variants for half-act rows
            qnd_sb = pers.tile([O, WIN], F16, name="qnd", tag="qnd")
            plnd_sb = pers.tile([O, BL], F32, name="plnd", tag="plnd")
            mbdT_sb = pers.tile([O, BL], F32, name="mbdT_sb", tag="mbdT_sb")
            r_sb = pers.tile([O, 2 * RW2], F16, name="r_sb", tag="r_sb")
            r_ps = rps_pool.tile([O, 2, RW2], F32, name="r_ps", tag="r_ps")

            # ---- proj matmuls, chunk-pipelined against the input DMA ----
            pps = [
                ps.tile([PT, WIN], F32, name=f"pps{t}", tag="ps") for t in range(NT)
            ]
            for c in range(NIN):
                for t in range(NT):
                    nc.tensor.matmul(
                        pps[t][:],
                        lhsT=wcat[:, c, PT * t : PT * (t + 1)],
                        rhs=xcat[:, c, :],
                        start=(c == 0),
                        stop=(c == NIN - 1),
                    )
            for t in range(NT):
                nc.scalar.copy(projTb[t][:], pps[t][:])
                nc.vector.tensor_copy(projL[t][:], pps[t][:, :BL])

            # ---- Q = sum_k projT over window; store -Q f16, -P f32 ----
            qps = ps.tile([O, WIN], F32, name="qps", tag="ps")
            for t in range(NT):
                nc.tensor.matmul(
                    qps[:],
                    lhsT=s2_sb[t],
                    rhs=projTb[t][:],
                    start=(t == 0),
                    stop=(t == NT - 1),
                )
            # qps = 2*Q -> qn = -Q (f16), pln = -P (f32, local cols)
            nc.vector.tensor_scalar(qn_sb[:], qps[:], -0.5, None, op0=ALU.mult)
            nc.vector.tensor_scalar(pln_sb[:], qps[:, :BL], -0.5, None, op0=ALU.mult)
            qpsd = ps.tile([O, WIN], F32, name="qpsd", tag="ps")
            for t in range(2):
                nc.tensor.matmul(
                    qpsd[:],
                    lhsT=s2_sb[t],
                    rhs=projTb[t][:],
                    start=(t == 0),
                    stop=(t == 1),
                )
            nc.vector.tensor_scalar(qnd_sb[:], qpsd[:], -0.5, None, op0=ALU.mult)
            nc.vector.tensor_scalar(
                plnd_sb[:], qpsd[:, :BL], -0.5, None, op0=ALU.mult
            )

            # ---- pairwise phase ----
            for g0 in range(0, BL, GSZ):
                gis = list(range(g0, min(g0 + GSZ, BL)))
                half = {i: i in HALF_ROWS for i in gis}
                offs = {i: BS * (i // BS) for i in gis}
                psums = {
                    i: ps.tile([O, EW], F32, name=f"ps{i}", tag="ps") for i in gis
                }
                aqs = {}
                for i in gis:
                    aq = work.tile([PT, NT, EW], F16, name=f"a{i}", tag="A")
                    off = offs[i]
                    for t in range(NT):
                        if half[i] and t >= 2:
                            nc.scalar.activation(
                                out=aq[:, t, :],
                                in_=projTb[t][:, off : off + EW],
                                func=AF.Abs,
                                bias=projL[t][:, i : i + 1],
                                scale=-1.0,
                            )
                        else:
                            nc.vector.tensor_scalar(
                                aq[:, t, :],
                                projTb[t][:, off : off + EW],
                                projL[t][:, i : i + 1],
                                0.0,
                                op0=ALU.subtract,
                                op1=ALU.max,
                            )
                    aqs[i] = aq
                # first row: row-major matmuls so its exp starts early;
                # remaining rows: w-outer (weight-stationary sweeps)
                def q_mm(i):
                    nc.tensor.matmul(
                        psums[i][:],
                        lhsT=eye_sb[:],
                        rhs=(qnd_sb if half[i] else qn_sb)[
                            :, offs[i] : offs[i] + EW
                        ],
                        start=True,
                        stop=False,
                    )

                def sel_mm(i, t):
                    sel = s1_sb[t] if (half[i] and t >= 2) else s2_sb[t]
                    nc.tensor.matmul(
                        psums[i][:],
                        lhsT=sel,
                        rhs=aqs[i][:, t, :],
                        start=False,
                        stop=(t == NT - 1),
                    )

                lead = gis[0]
                q_mm(lead)
                for t in range(NT):
                    sel_mm(lead, t)
                rest = gis[1:]
                for i in rest:
                    q_mm(i)
                for t in range(NT):
                    for i in rest:
                        sel_mm(i, t)
                for i in gis:
                    e_i = ework.tile([O, EW], F16, name=f"e{i}", tag="E")
                    nc.scalar.activation(
                        out=e_i[:],
                        in_=psums[i][:],
                        func=AF.Exp,
                        bias=(plnd_sb if half[i] else pln_sb)[:, i : i + 1],
                        scale=-1.0,
                        accum_out=mbdT_sb[:, i : i + 1],
                    )
                    # R[cls] += E[:, RRLO:RRHI] accumulated on the PE
                    cls = i // BS
                    nc.tensor.matmul(
                        r_ps[:, cls, :],
                        lhsT=eye_sb[:],
                        rhs=e_i[:, RRLO:RRHI],
                        start=(i % BS == 0),
                        stop=(i % BS == BS - 1),
                        skip_group_check=True,
                    )

            nc.vector.tensor_copy(r_sb[:], r_ps[:])
            nc.sync.dma_start(out=mbdT_d[:, :], in_=mbdT_sb[:])
            nc.sync.dma_start(out=r_d[:, :], in_=r_sb[:])
    nc.compile()
    return nc


_CACHE = {}


def _build_cached():
    if "nc" not in _CACHE:
        _CACHE["nc"] = build()
    return _CACHE["nc"]


def _selector(v: float) -> np.ndarray:
    sel = np.zeros((NT, PT, O), np.float32)
    for t in range(NT):
        for p in range(PT):
            sel[t, p, (t * PT + p) % O] = v
    return sel.astype(np.float16)


def make_in_maps(x: np.ndarray, W: np.ndarray):
    xT = np.ascontiguousarray(x.T.astype(np.float16))  # [IN, B]
    # k-major proj.T rows: row p corresponds to (o = p % O, k = p // O)
    perm = np.array([(p % O) * K + p // O for p in range(OK)], np.int64)
    wTk = np.ascontiguousarray(W.T.astype(np.float16)[:, perm])  # [IN, OK]
    sel2 = np.ascontiguousarray(_selector(2.0).transpose(1, 0, 2))
    sel1 = np.ascontiguousarray(_selector(1.0).transpose(1, 0, 2))
    eye = np.eye(O, dtype=np.float16)

    def prep(a, cols):
        return np.ascontiguousarray(
            a.reshape(NIN, 128, cols).transpose(1, 0, 2).reshape(128, NIN * cols)
        )

    wprep = prep(wTk, OK)
    in_maps = []
    for r in range(NCORES):
        cols = (BL * r + np.arange(WIN)) % B
        in_maps.append(
            {
                "xT": prep(np.ascontiguousarray(xT[:, cols]), WIN),
                "wT": wprep,
                "sel2": sel2,
                "sel1": sel1,
                "eye": eye,
            }
        )
    return in_maps


def run(x, W, trace=False, **kw):
    nc = _build_cached()
    in_maps = make_in_maps(x, W)
    return run_bass_kernel_spmd(
        nc, in_maps, core_ids=list(range(NCORES)), trace=trace, **kw
    )


def kernel(x: np.ndarray, W: np.ndarray) -> np.ndarray:
    x = np.asarray(x, np.float32)
    W = np.asarray(W, np.float32)
    res = run(x, W, trace=False)
    accums = [res.results[r]["mbdT"].astype(np.float32) for r in range(NCORES)]
    Rs = [
        res.results[r]["R"].astype(np.float32).reshape(O, 2, RW2)
        for r in range(NCORES)
    ]
    mbd = np.empty((B, O), np.float32)
    NB = B // BS  # 16 sub-blocks
    for m in range(NB):
        r, cls = m // 2, m % 2
        s = accums[r][:, BS * cls : BS * (cls + 1)].copy()  # [O, BS]
        for d in range(1, 8):
            c = (m - d) % NB
            rc, cc = c // 2, c % 2
            s += Rs[rc][:, cc, BS * (d - 1) : BS * d]
        mbd[BS * m : BS * (m + 1), :] = s.T
    mbd -= 1.0
    return np.concatenate([x, mbd], axis=1)


# revision 10
# speedup vs baseline: 1.0782x; 1.0062x over previous
"""MiniBatchDiscrimination Trainium2 kernel (symmetric half-window).

reference:
    proj = x @ W.T                      # [512, 500] -> [512, 100, 5]
    l1[i,j,o] = sum_k |proj[i,o,k] - proj[j,o,k]|
    mbd[i,o]  = sum_j exp(-l1[i,j,o]) - 1
    out = concat([x, mbd], axis=1)      # [512, 1124]

Strategy (8 cores, ~80.5us vs 122.8us for the full-j baseline):
  - Symmetry: l1/E are symmetric in (i,j), so core r computes its 64
    i-rows against only a 320-col cyclic j-window (own 64-block + the
    next 4 blocks; the distance-4 block is computed from both sides
    into own accums). Reciprocal sums R over window cols [64,256)
    (distance 1-3) serve the partner blocks; the host combines
    mbd[64r+i] = own_accum + sum_d R_{r-d}[64(d-1)+i] - 1. Work drops
    to 62.5% of the full 512-col pairwise at zero collective cost.
  - Absdiff without a sign-clear pass (walrus rejects fused abs):
    |d| = 2*relu(d) - d, and sum_k d telescopes to P_i[o] - Q[o,j]
    with Q = sum_k projT precomputed. DVE rows do 4 fused
    tensor_scalar(op0=subtract, op1=max 0) ops (2x mode); the PSUM
    gets 2*sum_k relu via a 2.0-selector matmul plus one identity
    matmul of -Q, and P_i folds into the exp bias for free.
  - 8 "half" rows per 64 rebalance DVE->Act: tiles 2,3 of their quad
    run as ScalarE Abs(-projT + col) with a 1.0-selector and a
    tiles-0,1-only Q correction (qnd/plnd).
  - exp + j-reduce: ScalarE Exp(scale=-1, bias=-P_i) reading PSUM,
    accum_out -> mbdT column; E written fp16 to SBUF.
  - R accumulation on the PE: identity-matmul of E[:, 64:256] into a
    dedicated PSUM bank (start at i=0, stop at i=63), freeing DVE.
  - Input DMA as 2-chunk pairs issued early from the Sync queue and
    overlapped with the projection matmuls; selectors sent host-side
    transposed ([PT, NT, O]) so each loads in one descriptor sweep.
  - GpSimd is left idle on purpose: its Q7 tensor ops are ~10x slower
    than the cost model suggests and their SBUF traffic slows every
    other engine (measured 222us with subs offloaded there).
"""

import sys

import numpy as np

sys.path.insert(0, "/opt/trn_rl_repo")

import concourse.bacc as bacc  # noqa: E402
import concourse.mybir as mybir  # noqa: E402
import concourse.tile as tile  # noqa: E402
from concourse.bass_utils import run_bass_kernel_spmd  # noqa: E402

B, IN, O, K = 512, 1024, 100, 5
OK = O * K  # 500
NCORES = 8
BL = B // NCORES  # 64 local rows per core
WIN = 5 * BL  # 320 window cols per core
RLO, RHI = BL, 4 * BL  # reciprocal cols [64, 256)
RW = RHI - RLO  # 192
NT = 4  # proj.T partition tiles
PT = OK // NT  # 125 partitions per tile
NIN = IN // 128  # 8 contraction chunks

F32 = mybir.dt.float32
F16 = mybir.dt.float16
AF = mybir.ActivationFunctionType
ALU = mybir.AluOpType

GSZ = 7  # i-rows per PSUM group (7 banks; 1 bank holds R)
HALF_ROWS = frozenset(i for i in range(BL) if i % 5 == 2)
BS = 32  # sub-block size (16 blocks of 32 over B)
EW = 288  # effective window per row (9 sub-blocks)
RRLO, RRHI = 32, 256  # R region, relative to the row's 288-window
RW2 = RRHI - RRLO  # 224


def build():
    nc = bacc.Bacc("TRN2", target_bir_lowering=False)
    xT_d = nc.dram_tensor("xT", [128, NIN * WIN], F16, kind="ExternalInput")
    wT_d = nc.dram_tensor("wT", [128, NIN * OK], F16, kind="ExternalInput")
    sel2_d = nc.dram_tensor("sel2", [PT, NT, O], F16, kind="ExternalInput")
    sel1_d = nc.dram_tensor("sel1", [PT, NT, O], F16, kind="ExternalInput")
    eye_d = nc.dram_tensor("eye", [O, O], F16, kind="ExternalInput")
    mbdT_d = nc.dram_tensor("mbdT", [O, BL], F32, kind="ExternalOutput")
    r_d = nc.dram_tensor("R", [O, 2 * RW2], F16, kind="ExternalOutput")

    with tile.TileContext(nc) as tc:
        with (
            tc.tile_pool(name="pers", bufs=1) as pers,
            tc.tile_pool(name="io", bufs=1) as io,
            tc.tile_pool(name="work", bufs=16) as work,
            tc.tile_pool(name="ework", bufs=8) as ework,
            tc.tile_pool(name="ps", bufs=7, space="PSUM") as ps,
            tc.tile_pool(name="rps", bufs=1, space="PSUM") as rps_pool,
        ):
            # ---- input DMA: paired chunks, issue split across sync+scalar
            xcat = io.tile([128, NIN, WIN], F16, name="xcat", tag="xcat")
            wcat = io.tile([128, NIN, OK], F16, name="wcat", tag="wcat")
            for c2 in range(NIN // 2):
                nc.sync.dma_start(
                    out=xcat[:, 2 * c2 : 2 * c2 + 2, :],
                    in_=xT_d[:, 2 * c2 * WIN : (2 * c2 + 2) * WIN],
                )
                nc.scalar.dma_start(
                    out=wcat[:, 2 * c2 : 2 * c2 + 2, :],
                    in_=wT_d[:, 2 * c2 * OK : (2 * c2 + 2) * OK],
                )
            s2_all = pers.tile([PT, NT, O], F16, name="s2a", tag="s2a")
            s1_all = pers.tile([PT, NT, O], F16, name="s1a", tag="s1a")
            eye_sb = pers.tile([O, O], F16, name="eye", tag="eye")
            nc.sync.dma_start(out=s2_all[:], in_=sel2_d[:, :, :])
            nc.scalar.dma_start(out=s1_all[:], in_=sel1_d[:, :, :])
            nc.sync.dma_start(out=eye_sb[:], in_=eye_d[:, :])
            s2_sb = [s2_all[:, t, :] for t in range(NT)]
            s1_sb = [s1_all[:, t, :] for t in range(NT)]

            projTb = [
                pers.tile([PT, WIN], F16, name=f"projTb{t}", tag=f"projTb{t}")
                for t in range(NT)
            ]
            projL = [
                pers.tile([PT, BL], F32, name=f"projL{t}", tag=f"projL{t}")
                for t in range(NT)
            ]
            qn_sb = pers.tile([O, WIN], F16, name="qn", tag="qn")  # -Q fp16
            pln_sb = pers.tile([O, BL], F32, name="pln", tag="pln")  # -P_i f32
            # subset (tiles 0,1) variants for half-act rows
            qnd_sb = pers.tile([O, WIN], F16, name="qnd", tag="qnd")
            plnd_sb = pers.tile([O, BL], F32, name="plnd", tag="plnd")
            mbdT_sb = pers.tile([O, BL], F32, name="mbdT_sb", tag="mbdT_sb")
            r_sb = pers.tile([O, 2 * RW2], F16, name="r_sb", tag="r_sb")
            r_ps = rps_pool.tile([O, 2, RW2], F32, name="r_ps", tag="r_ps")

            # ---- proj matmuls, chunk-pipelined against the input DMA ----
            pps = [
                ps.tile([PT, WIN], F32, name=f"pps{t}", tag="ps") for t in range(NT)
            ]
            for c in range(NIN):
                for t in range(NT):
                    nc.tensor.matmul(
                        pps[t][:],
                        lhsT=wcat[:, c, PT * t : PT * (t + 1)],
                        rhs=xcat[:, c, :],
                        start=(c == 0),
                        stop=(c == NIN - 1),
                    )
            for t in range(NT):
                nc.scalar.copy(projTb[t][:], pps[t][:])
                nc.vector.tensor_copy(projL[t][:], pps[t][:, :BL])

            # ---- Q = sum_k projT over window; store -Q f16, -P f32 ----
            qps = ps.tile([O, WIN], F32, name="qps", tag="ps")
            for t in range(NT):
                nc.tensor.matmul(
                    qps[:],
                    lhsT=s2_sb[t],
                    rhs=projTb[t][:],
                    start=(t == 0),
                    stop=(t == NT - 1),
                )
            # qps = 2*Q -> qn = -Q (f16), pln = -P (f32, local cols)
            nc.vector.tensor_scalar(qn_sb[:], qps[:], -0.5, None, op0=ALU.mult)
            nc.vector.tensor_scalar(pln_sb[:], qps[:, :BL], -0.5, None, op0=ALU.mult)
            qpsd = ps.tile([O, WIN], F32, name="qpsd", tag="ps")
            for t in range(2):
                nc.tensor.matmul(
                    qpsd[:],
                    lhsT=s2_sb[t],
                    rhs=projTb[t][:],
                    start=(t == 0),
                    stop=(t == 1),
                )
            nc.vector.tensor_scalar(qnd_sb[:], qpsd[:], -0.5, None, op0=ALU.mult)
            nc.vector.tensor_scalar(
                plnd_sb[:], qpsd[:, :BL], -0.5, None, op0=ALU.mult
            )

            # ---- pairwise phase ----
            for g0 in range(0, BL, GSZ):
                gis = list(range(g0, min(g0 + GSZ, BL)))
                half = {i: i in HALF_ROWS for i in gis}
                offs = {i: BS * (i // BS) for i in gis}
                psums = {
                    i: ps.tile([O, EW], F32, name=f"ps{i}", tag="ps") for i in gis
                }
                aqs = {}
                for i in gis:
                    aq = work.tile([PT, NT, EW], F16, name=f"a{i}", tag="A")
                    off = offs[i]
                    for t in range(NT):
                        if half[i] and t >= 2:
                            nc.scalar.activation(
                                out=aq[:, t, :],
                                in_=projTb[t][:, off : off + EW],
                                func=AF.Abs,
                                bias=projL[t][:, i : i + 1],
                                scale=-1.0,
                            )
                        else:
                            nc.vector.tensor_scalar(
                                aq[:, t, :],
                                projTb[t][:, off : off + EW],
                                projL[t][:, i : i + 1],
                                0.0,
                                op0=ALU.subtract,
                                op1=ALU.max,
                            )
                    aqs[i] = aq
                # first row: row-major matmuls so its exp starts early;
                # remaining rows: w-outer (weight-stationary sweeps)
                def q_mm(i):
                    nc.tensor.matmul(
                        psums[i][:],
                        lhsT=eye_sb[:],
                        rhs=(qnd_sb if half[i] else qn_sb)[
                            :, offs[i] : offs[i] + EW
                        ],
                        start=True,
                        stop=False,
                    )

                def sel_mm(i, t):
                    sel = s1_sb[t] if (half[i] and t >= 2) else s2_sb[t]
                    nc.tensor.matmul(
                        psums[i][:],
                        lhsT=sel,
                        rhs=aqs[i][:, t, :],
                        start=False,
                        stop=(t == NT - 1),
                    )

                lead = gis[0]
                q_mm(lead)
                for t in range(NT):
                    sel_mm(lead, t)
                rest = gis[1:]
                for i in rest:
                    q_mm(i)
                for t in range(NT):
                    for i in rest:
                        sel_mm(i, t)
                for i in gis:
                    e_i = ework.tile([O, EW], F16, name=f"e{i}", tag="E")
                    nc.scalar.activation(
                        out=e_i[:],
                        in_=psums[i][:],
                        func=AF.Exp,
                        bias=(plnd_sb if half[i] else pln_sb)[:, i : i + 1],
                        scale=-1.0,
                        accum_out=mbdT_sb[:, i : i + 1],
                    )
                    # R[cls] += E[:, RRLO:RRHI] accumulated on the PE
                    cls = i // BS
                    nc.tensor.matmul(
                        r_ps[:, cls, :],
                        lhsT=eye_sb[:],
                        rhs=e_i[:, RRLO:RRHI],
                        start=(i % BS == 0),
                        stop=(i % BS == BS - 1),
                        skip_group_check=True,
                    )

            nc.vector.tensor_copy(r_sb[:], r_ps[:])
            nc.sync.dma_start(out=mbdT_d[:, :], in_=mbdT_sb[:])
            nc.sync.dma_start(out=r_d[:, :], in_=r_sb[:])
    nc.compile()
    return nc


_CACHE = {}


def _build_cached():
    if "nc" not in _CACHE:
        _CACHE["nc"] = build()
    return _CACHE["nc"]


def _selector(v: float) -> np.ndarray:
    sel = np.zeros((NT, PT, O), np.float32)
    for t in range(NT):
        for p in range(PT):
            sel[t, p, (t * PT + p) % O] = v
    return sel.astype(np.float16)


def make_in_maps(x: np.ndarray, W: np.ndarray):
    xT = np.ascontiguousarray(x.T.astype(np.float16))  # [IN, B]
    # k-major proj.T rows: row p corresponds to (o = p % O, k = p // O)
    perm = np.array([(p % O) * K + p // O for p in range(OK)], np.int64)
    wTk = np.ascontiguousarray(W.T.astype(np.float16)[:, perm])  # [IN, OK]
    sel2 = np.ascontiguousarray(_selector(2.0).transpose(1, 0, 2))
    sel1 = np.ascontiguousarray(_selector(1.0).transpose(1, 0, 2))
    eye = np.eye(O, dtype=np.float16)

    def prep(a, cols):
        return np.ascontiguousarray(
            a.reshape(NIN, 128, cols).transpose(1, 0, 2).reshape(128, NIN * cols)
        )

    wprep = prep(wTk, OK)
    in_maps = []
    for r in range(NCORES):
        cols = (BL * r + np.arange(WIN)) % B
        in_maps.append(
            {
                "xT": prep(np.ascontiguousarray(xT[:, cols]), WIN),
                "wT": wprep,
                "sel2": sel2,
                "sel1": sel1,
                "eye": eye,
            }
        )
    return in_maps


def run(x, W, trace=False, **kw):
    nc = _build_cached()
    in_maps = make_in_maps(x, W)
    return run_bass_kernel_spmd(
        nc, in_maps, core_ids=list(range(NCORES)), trace=trace, **kw
    )


def kernel(x: np.ndarray, W: np.ndarray) -> np.ndarray:
    x = np.asarray(x, np.float32)
    W = np.asarray(W, np.float32)
    res = run(x, W, trace=False)
    accums = [res.results[r]["mbdT"].astype(np.float32) for r in range(NCORES)]
    Rs = [
        res.results[r]["R"].astype(np.float32).reshape(O, 2, RW2)
        for r in range(NCORES)
    ]
    mbd = np.empty((B, O), np.float32)
    NB = B // BS  # 16 sub-blocks
    for m in range(NB):
        r, cls = m // 2, m % 2
        s = accums[r][:, BS * cls : BS * (cls + 1)].copy()  # [O, BS]
        for d in range(1, 8):
            c = (m - d) % NB
            rc, cc = c // 2, c % 2
            s += Rs[rc][:, cc, BS * (d - 1) : BS * d]
        mbd[BS * m : BS * (m + 1), :] = s.T
    mbd -= 1.0
    return np.concatenate([x, mbd], axis=1)
